# revision 14
# baseline (speedup 1.0000x reference)
"""Trainium2 Bass kernel for nn_AtomBlock (sparse windowed attention atom block).

Sharding: 8 cores = 2 batches x 4 row-chunks of 1024 atoms. Each core computes
its 1024 atoms end-to-end with a +-16-row halo for the windowed attention.

Self-contained: hardcodes all shapes; builds/compiles the Bass program once per
process and runs it SPMD on 8 NeuronCores via run_bass_kernel_spmd.
"""
import numpy as np

import concourse.bass as bass
import concourse.bacc as bacc
import concourse.tile as tile
from concourse import mybir
from concourse.bass_utils import run_bass_kernel_spmd

F32 = mybir.dt.float32
F32R = mybir.dt.float32r
F16 = mybir.dt.float16
I32 = mybir.dt.int32
AL = mybir.AluOpType
AF = mybir.ActivationFunctionType
AX = mybir.AxisListType
IOA = bass.IndirectOffsetOnAxis

B, N, D, H, DH, WIN = 2, 4096, 128, 4, 32, 16
NT, DM, NP = 1024, 512, 16384
RC = 1024              # rows per core (chunk)
NH = 1152              # halo rows per core (9 chunks of 128)
NBLK = 8               # central 128-row blocks per core
JW = 160               # score-tile j extent per block
NSLOT = NBLK * JW * D  # 163840 banded bias slots per core
NG = 2048              # pair groups (8 pairs each)
EXP_SHIFT = -8.0       # exp(score-8): fp16-safe, cancels in softmax

_PROG = None


def _build_program():
    nc = bacc.Bacc("TRN2", target_bir_lowering=False, debug=False, num_devices=8)

    def din(name, shape, dt=F32):
        return nc.dram_tensor(name, list(shape), dt, kind="ExternalInput").ap()

    io = {}
    io["qh"] = din("qh", (128, 9, D))          # halo q rows-part [p, r, d]
    io["qc"] = din("qc", (128, NBLK, D))       # central q block-part [il, I, d]
    io["hc"] = din("hc", (NT, DM))             # h_cond[b] (gather source)
    io["tok"] = din("tok", (128, 9), I32)      # gather row indices per halo chunk
    io["tv"] = din("tv", (128, 1))             # t_emb[b]
    io["pi"] = din("pi", (128, 128), I32)      # pair idx0  [p, c] = pair p*128+c
    io["pj"] = din("pj", (128, 128), I32)
    io["pv"] = din("pv", (128, 128), I32)      # pair_valid as int
    io["plm"] = din("plm", (128, 128, 16))
    io["bandi"] = din("bandi", (NSLOT, 4))     # band mask init; scatter target
    io["rvec"] = din("rvec", (128, 1), I32)    # r0 per core
    io["pad"] = din("pad", (128, NBLK))        # atom_pad_mask central, block-part
    io["ut"] = din("ut", (128, 128))           # strict upper triangular ones
    io["iota"] = din("iota", (128, 128))       # iota[p, c] = c
    io["gval"] = din("gval", (128, 16))        # gval[p, k] = p*16+k
    io["idn"] = din("idn", (128, 128))         # identity
    # weights (host pre-transposed)
    io["cwT"] = din("cwT", (DM, D))            # cond_w.T
    io["cb"] = din("cb", (128, 1))             # cond_b
    for s in ("s1", "s2"):
        io[s + "sw"] = din(s + "sw", (D, D))   # (scale_w * s_norm_w).T
        io[s + "sb"] = din(s + "sb", (128, 1))
        io[s + "bw"] = din(s + "bw", (D, D))   # (bias_w * s_norm_w).T
    for w in ("wqT", "wkT", "wvT", "wgT", "woT", "agT", "tgT"):
        io[w] = din(w, (D, D))
    io["wqb"] = din("wqb", (128, 1))
    io["agb"] = din("agb", (128, 128))         # attn_gate_b bcast along partitions
    io["tgb"] = din("tgb", (128, 128))
    io["w1T"] = din("w1T", (D, 4 * D))
    io["w3T"] = din("w3T", (D, 4 * D))
    io["w2T"] = din("w2T", (4 * D, D))
    io["pbw"] = din("pbw", (128, H, 16))       # pair_bias_w*ln_w bcast over partitions
    io["pbs"] = din("pbs", (128, H))           # per-head sum(pair_bias_w*ln_w)
    io["pbc"] = din("pbc", (128, H))           # per-head sum(pair_bias_w*ln_b)

    out_d = nc.dram_tensor("out", [128, NBLK, D], F32, kind="ExternalOutput").ap()
    payred_d = nc.dram_tensor("payred_d", [NG, 8], F32)

    with tile.TileContext(nc) as tc:
        _emit(nc, tc, io, out_d, payred_d)

    nc.compile()
    return nc


def _emit(nc, tc, io, out_d, payred_d):
    STAGE = 9
    import contextlib
    ctx = contextlib.ExitStack()
    sb = ctx.enter_context(tc.tile_pool(name="sb", bufs=1))
    sb2 = ctx.enter_context(tc.tile_pool(name="sb2", bufs=2))
    pj = ctx.enter_context(tc.tile_pool(name="pjp", bufs=3, space="PSUM"))
    psA = ctx.enter_context(tc.tile_pool(name="psA", bufs=2, space="PSUM"))
    psB = ctx.enter_context(tc.tile_pool(name="psB", bufs=2, space="PSUM"))
    psO = ctx.enter_context(tc.tile_pool(name="psO", bufs=1, space="PSUM"))

    def full(ap):
        return ap[tuple(slice(None) for _ in ap.shape)]

    def load(name, shape=None, dt=F32, pool=sb):
        t = pool.tile(list(shape) if shape else list(io[name].shape), dt, name=name + "_s")
        nc.sync.dma_start(out=full(t), in_=full(io[name]))
        return t

    # ---------------- constant / weight loads ----------------
    idn = load("idn")
    ut = load("ut")
    iota = load("iota")
    gval_f = load("gval")
    gval = sb.tile([128, 16], F16)
    nc.vector.tensor_copy(out=gval[:], in_=gval_f[:])
    rvec = load("rvec", dt=I32)
    pad = load("pad")
    tv = load("tv")
    cbv = load("cb")
    cb2 = sb.tile([128, 1], F32)
    nc.vector.tensor_tensor(out=cb2[:], in0=tv[:], in1=cbv[:], op=AL.add)
    eps_c = sb.tile([128, 1], F32)
    nc.vector.memset(eps_c[:], 1e-5)
    shf_c = sb.tile([128, 1], F32)
    nc.vector.memset(shf_c[:], EXP_SHIFT)

    def wload(name, shape, dt):
        f = sb2.tile([128, 512], F32, name=name + "_f", tag="stg")
        fs = f[tuple(slice(0, s) for s in ((shape[0],) + tuple(shape[1:])))] if len(shape) == 2 else f
        fs = f[0:shape[0], 0:shape[1]]
        nc.sync.dma_start(out=fs, in_=full(io[name]))
        t = sb.tile(list(shape), dt, name=name + "_r")
        nc.vector.tensor_copy(out=full(t), in_=fs)
        return t

    cwT = sb2.tile([128, 512], F32, name="cwT_f", tag="stg")
    nc.sync.dma_start(out=cwT[:].rearrange("p (k d) -> p k d", k=4),
                      in_=io["cwT"][:, :].rearrange("(k p) d -> p k d", p=128))
    cwTr = sb.tile([128, 4, D], F32R, name="cwT_r")
    nc.vector.tensor_copy(out=cwTr[:], in_=cwT[:].rearrange("p (k d) -> p k d", k=4))
    s1sw = wload("s1sw", (D, D), F32R)
    s1bw = wload("s1bw", (D, D), F32R)
    s2sw = wload("s2sw", (D, D), F32R)
    s2bw = wload("s2bw", (D, D), F32R)
    s1sb = load("s1sb")
    s2sb = load("s2sb")
    wqT = wload("wqT", (D, D), F16)
    wkT = wload("wkT", (D, D), F16)
    wvT = wload("wvT", (D, D), F16)
    wgT = wload("wgT", (D, D), F16)
    woT = wload("woT", (D, D), F16)
    wqb = load("wqb")
    agb = load("agb")
    tgb = load("tgb")
    w1T = wload("w1T", (D, 4 * D), F16)
    w3T = wload("w3T", (D, 4 * D), F16)
    w2Tf = sb2.tile([128, 512], F32, name="w2T_f", tag="stg")
    nc.sync.dma_start(out=w2Tf[:].rearrange("p (k d) -> p k d", k=4),
                      in_=io["w2T"][:, :].rearrange("(k p) d -> p k d", p=128))
    w2T = sb.tile([128, 4, D], F16, name="w2T_h")
    nc.vector.tensor_copy(out=w2T[:], in_=w2Tf[:].rearrange("p (k d) -> p k d", k=4))

    # ================= pair path =================
    pi = load("pi", dt=I32)
    pjx = load("pj", dt=I32)
    pv = load("pv", dt=I32)
    plm = load("plm")
    pbw = load("pbw")
    pbs = load("pbs")
    pbc = load("pbc")

    u_ = sb.tile([128, 128], I32)
    nc.vector.tensor_tensor(out=u_[:], in0=pjx[:], in1=pi[:], op=AL.subtract)
    nc.vector.tensor_scalar(out=u_[:], in0=u_[:], scalar1=16, scalar2=None, op0=AL.add)
    irel = sb.tile([128, 128], I32)
    nc.vector.tensor_tensor(out=irel[:], in0=pi[:], in1=rvec[:, :1].to_broadcast([128, 128]), op=AL.subtract)
    s_a = sb.tile([128, 128], I32)
    flg = sb.tile([128, 128], I32)
    nc.vector.tensor_scalar(out=flg[:], in0=u_[:], scalar1=0, scalar2=None, op0=AL.is_ge)
    nc.vector.tensor_scalar(out=s_a[:], in0=u_[:], scalar1=32, scalar2=None, op0=AL.is_le)
    nc.vector.tensor_tensor(out=flg[:], in0=flg[:], in1=s_a[:], op=AL.logical_and)
    nc.vector.tensor_scalar(out=s_a[:], in0=irel[:], scalar1=0, scalar2=None, op0=AL.is_ge)
    nc.vector.tensor_tensor(out=flg[:], in0=flg[:], in1=s_a[:], op=AL.logical_and)
    nc.vector.tensor_scalar(out=s_a[:], in0=irel[:], scalar1=RC - 1, scalar2=None, op0=AL.is_le)
    nc.vector.tensor_tensor(out=flg[:], in0=flg[:], in1=s_a[:], op=AL.logical_and)
    nc.vector.tensor_scalar(out=s_a[:], in0=pv[:], scalar1=1, scalar2=None, op0=AL.is_ge)
    nc.vector.tensor_tensor(out=flg[:], in0=flg[:], in1=s_a[:], op=AL.logical_and)
    il_ = sb.tile([128, 128], I32)
    nc.vector.tensor_scalar(out=il_[:], in0=irel[:], scalar1=127, scalar2=None, op0=AL.bitwise_and)
    slot = sb.tile([128, 128], I32)
    nc.vector.tensor_scalar(out=s_a[:], in0=irel[:], scalar1=7, scalar2=None, op0=AL.logical_shift_right)
    nc.vector.tensor_scalar(out=slot[:], in0=s_a[:], scalar1=JW * D, scalar2=None, op0=AL.mult)
    nc.vector.tensor_tensor(out=s_a[:], in0=u_[:], in1=il_[:], op=AL.add)
    nc.vector.tensor_scalar(out=s_a[:], in0=s_a[:], scalar1=D, scalar2=None, op0=AL.mult)
    nc.vector.tensor_tensor(out=slot[:], in0=slot[:], in1=s_a[:], op=AL.add)
    nc.vector.tensor_tensor(out=slot[:], in0=slot[:], in1=il_[:], op=AL.add)
    flg_f = sb.tile([128, 128], F32)
    nc.vector.tensor_copy(out=flg_f[:], in_=flg[:])
    slot_f = sb.tile([128, 128], F32)
    nc.vector.tensor_copy(out=slot_f[:], in_=slot[:])

    # LN(p_lm) + per-head bias (algebraically folded)
    mu_p = sb.tile([128, 128], F32)
    nc.vector.tensor_reduce(out=mu_p[:], in_=plm[:], axis=AX.X, op=AL.add)
    nc.vector.tensor_scalar(out=mu_p[:], in0=mu_p[:], scalar1=1.0 / 16, scalar2=None, op0=AL.mult)
    sq_p = sb2.tile([128, 128, 16], F32, name="sq_p", tag="th", bufs=1)
    nc.gpsimd.tensor_tensor(out=sq_p[:], in0=plm[:], in1=plm[:], op=AL.mult)
    ssq_p = sb.tile([128, 128], F32)
    nc.vector.tensor_reduce(out=ssq_p[:], in_=sq_p[:], axis=AX.X, op=AL.add)
    var_p = sb.tile([128, 128], F32)
    nc.vector.tensor_scalar(out=var_p[:], in0=ssq_p[:], scalar1=1.0 / 16, scalar2=None, op0=AL.mult)
    mu2 = sb.tile([128, 128], F32)
    nc.vector.tensor_tensor(out=mu2[:], in0=mu_p[:], in1=mu_p[:], op=AL.mult)
    nc.vector.tensor_tensor(out=var_p[:], in0=var_p[:], in1=mu2[:], op=AL.subtract)
    rr_p = sb.tile([128, 128], F32)
    nc.scalar.activation(out=rr_p[:], in_=var_p[:], func=AF.Sqrt, bias=eps_c[:, :1], scale=1.0)
    nc.vector.reciprocal(out=rr_p[:], in_=rr_p[:])

    pay = sb.tile([128, 128, 8], F32)
    nc.vector.memset(pay[:], 0.0)
    for h in range(H):
        th = sb2.tile([128, 128, 16], F32, name="th", tag="th", bufs=1)
        nc.gpsimd.tensor_tensor(
            out=th[:], in0=plm[:],
            in1=pbw[:, h:h + 1, :].to_broadcast([128, 128, 16]), op=AL.mult)
        sh = sb2.tile([128, 128], F32, name="sh", tag="sh")
        nc.vector.tensor_reduce(out=sh[:], in_=th[:], axis=AX.X, op=AL.add)
        t2 = sb2.tile([128, 128], F32, name="t2h", tag="t2h")
        nc.vector.tensor_scalar(out=t2[:], in0=mu_p[:], scalar1=pbs[:, h:h + 1], scalar2=None, op0=AL.mult)
        nc.vector.tensor_tensor(out=sh[:], in0=sh[:], in1=t2[:], op=AL.subtract)
        nc.vector.tensor_tensor(out=sh[:], in0=sh[:], in1=rr_p[:], op=AL.mult)
        nc.vector.tensor_scalar(out=pay[:, :, h], in0=sh[:], scalar1=pbc[:, h:h + 1], scalar2=None, op0=AL.add)
    nc.vector.tensor_scalar(out=pay[:, :, 4], in0=slot_f[:], scalar1=1.0, scalar2=None, op0=AL.add)
    fpay = sb.tile([128, 128, 8], F32)
    nc.gpsimd.tensor_tensor(
        out=fpay[:], in0=pay[:],
        in1=flg_f[:].rearrange("p (c o) -> p c o", o=1).to_broadcast([128, 128, 8]), op=AL.mult)
    payred = sb.tile([128, 16, 8], F32)
    nc.vector.tensor_reduce(out=payred[:], in_=fpay[:].rearrange("p (j k) f -> p k f j", j=8),
                            axis=AX.X, op=AL.add)
    cntg = sb.tile([128, 16], F32)
    nc.vector.tensor_reduce(out=cntg[:], in_=flg_f[:].rearrange("p (j k) -> p k j", j=8),
                            axis=AX.X, op=AL.add)
    nc.sync.dma_start(out=payred_d[:, :].rearrange("(p k) f -> p k f", p=128), in_=payred[:])

    flag_g = sb.tile([128, 16], F32)
    nc.vector.tensor_scalar(out=flag_g[:], in0=cntg[:], scalar1=1.0, scalar2=None, op0=AL.min)
    zer16 = sb.tile([128, 16], F32)
    nc.vector.memset(zer16[:], 0.0)
    incl = sb.tile([128, 16], F32)
    nc.vector.tensor_tensor_scan(out=incl[:], data0=flag_g[:], data1=zer16[:],
                                 initial=0.0, op0=AL.add, op1=AL.add)
    tot = sb.tile([128, 1], F32)
    nc.vector.tensor_copy(out=tot[:], in_=incl[:, 15:16])
    pp_ps = pj.tile([128, 512], F32, space="PSUM", name="pp_ps", tag="pj")
    nc.tensor.matmul(out=pp_ps[:, 0:1], lhsT=ut[:], rhs=tot[:], start=True, stop=True)
    Pp = sb.tile([128, 1], F32)
    nc.vector.tensor_copy(out=Pp[:], in_=pp_ps[:, 0:1])
    rank = sb.tile([128, 16], F32)
    nc.vector.tensor_scalar(out=rank[:], in0=incl[:], scalar1=Pp[:, :1], scalar2=None, op0=AL.add)
    nc.vector.tensor_tensor(out=rank[:], in0=rank[:], in1=flag_g[:], op=AL.subtract)
    rmask = sb.tile([128, 16], F32)
    nc.vector.tensor_tensor(out=rmask[:], in0=rank[:], in1=flag_g[:], op=AL.mult)
    tm = sb.tile([128, 16], F32)
    nc.vector.tensor_scalar(out=tm[:], in0=flag_g[:], scalar1=-200.0, scalar2=200.0, op0=AL.mult, op1=AL.add)
    nc.vector.tensor_tensor(out=rmask[:], in0=rmask[:], in1=tm[:], op=AL.add)
    inv_ps = pj.tile([128, 512], F32, space="PSUM", name="inv_ps", tag="pj")
    for k in range(16):
        oh = sb2.tile([128, 128], F16, name="oh", tag="oh")
        nc.vector.tensor_scalar(out=oh[:], in0=iota[:], scalar1=rmask[:, k:k + 1], scalar2=None, op0=AL.is_equal)
        nc.tensor.matmul(out=inv_ps[:, 0:1], lhsT=oh[:], rhs=gval[:, k:k + 1],
                         start=(k == 0), stop=(k == 15))
    goff_f = sb.tile([128, 1], F32)
    nc.vector.tensor_copy(out=goff_f[:], in_=inv_ps[:, 0:1])
    goff = sb.tile([128, 1], I32)
    nc.vector.tensor_copy(out=goff[:], in_=goff_f[:])
    cmp_sb = sb.tile([128, 8], F32)
    nc.gpsimd.indirect_dma_start(out=cmp_sb[:], out_offset=None, in_=payred_d[:, :],
                                 in_offset=IOA(ap=goff[:, :1], axis=0))
    offf = sb.tile([128, 1], F32)
    nc.vector.tensor_scalar(out=offf[:], in0=cmp_sb[:, 4:5], scalar1=float(NSLOT + 1), scalar2=-1.0,
                            op0=AL.min, op1=AL.add)
    zf = sb.tile([128, 1], F32)
    nc.vector.tensor_scalar(out=zf[:], in0=cmp_sb[:, 4:5], scalar1=0.0, scalar2=float(NSLOT + 1),
                            op0=AL.is_equal, op1=AL.mult)
    nc.vector.tensor_tensor(out=offf[:], in0=offf[:], in1=zf[:], op=AL.add)
    offi = sb.tile([128, 1], I32)
    nc.vector.tensor_copy(out=offi[:], in_=offf[:])
    vals4 = sb.tile([128, 4], F32)
    nc.vector.tensor_copy(out=vals4[:], in_=cmp_sb[:, 0:4])
    nc.gpsimd.indirect_dma_start(out=io["bandi"][:, :],
                                 out_offset=IOA(ap=offi[:, :1], axis=0),
                                 in_=vals4[:], in_offset=None,
                                 bounds_check=NSLOT - 1, oob_is_err=False)

    if STAGE < 1:
        outp0 = sb.tile([128, NBLK, D], F32, name="outp0")
        nc.vector.memset(outp0[:], 0.0)
        nc.sync.dma_start(out=out_d[:, :, :], in_=outp0[:])
        ctx.close()
        return

    # ================= main path =================
    tok = load("tok", dt=I32)
    h_T = [sb.tile([128, NH], F32R, name=f"hT{k}") for k in range(4)]
    for r in range(9):
        hg = sb2.tile([128, DM], F32, name="hg", tag="hg")
        nc.gpsimd.indirect_dma_start(out=hg[:], out_offset=None, in_=io["hc"][:, :],
                                     in_offset=IOA(ap=tok[:, r:r + 1], axis=0))
        for k in range(4):
            tp = psA.tile([128, 128], F32, space="PSUM", name="htp", tag="sA")
            nc.tensor.transpose(out=tp[:], in_=hg[:, 128 * k:128 * (k + 1)], identity=idn[:])
            nc.scalar.activation(out=h_T[k][:, 128 * r:128 * (r + 1)], in_=tp[:],
                                 func=AF.Copy, bias=0.0, scale=1.0)

    # cond_T = cwT.T-chunks @ h_T + (t_emb + cond_b)
    cond_T = sb.tile([128, NH], F32, name="cond_T")
    cond_Tr = sb.tile([128, NH], F32R, name="cond_Tr")
    for t in range(3):
        sl = slice(384 * t, 384 * (t + 1))
        cps = pj.tile([128, 512], F32, space="PSUM", name="cps", tag="pj")
        for k in range(4):
            nc.tensor.matmul(out=cps[:, 0:384], lhsT=cwTr[:, k, :], rhs=h_T[k][:, sl],
                             start=(k == 0), stop=(k == 3))
        nc.scalar.activation(out=cond_T[:, sl], in_=cps[:, 0:384], func=AF.Identity,
                             bias=cb2[:, :1], scale=1.0)
        nc.vector.tensor_copy(out=cond_Tr[:, sl], in_=cond_T[:, sl])

    # LN over features for a feat-part tensor: transpose -> rows-part stats ->
    # apply -> transpose back (f32r out)
    def ln_featpart(src_T, nchunks, dst_T):
        xr = sb.tile([128, nchunks, D], F32, name="lnxr", tag="ln_xr")
        for r in range(nchunks):
            tp = psA.tile([128, 128], F32, space="PSUM", name="lntp", tag="sA")
            nc.tensor.transpose(out=tp[:], in_=src_T[:, 128 * r:128 * (r + 1)], identity=idn[:])
            nc.scalar.activation(out=xr[:, r, :], in_=tp[:], func=AF.Copy, bias=0.0, scale=1.0)
        mu = sb.tile([128, nchunks], F32, name="lnmu", tag="ln_mu")
        nc.vector.tensor_reduce(out=mu[:], in_=xr[:], axis=AX.X, op=AL.add)
        nc.vector.tensor_scalar(out=mu[:], in0=mu[:], scalar1=1.0 / D, scalar2=None, op0=AL.mult)
        xc = sb.tile([128, nchunks, D], F32, name="lnxc", tag="ln_xc")
        nc.vector.tensor_tensor(out=xc[:], in0=xr[:],
                                in1=mu[:].rearrange("p (c o) -> p c o", o=1).to_broadcast([128, nchunks, D]),
                                op=AL.subtract)
        sq = sb.tile([128, nchunks, D], F32, name="lnsq", tag="ln_sq")
        nc.vector.tensor_tensor(out=sq[:], in0=xc[:], in1=xc[:], op=AL.mult)
        var = sb.tile([128, nchunks], F32, name="lnvar", tag="ln_var")
        nc.vector.tensor_reduce(out=var[:], in_=sq[:], axis=AX.X, op=AL.add)
        sd = sb.tile([128, nchunks], F32, name="lnsd", tag="ln_sd")
        nc.scalar.activation(out=sd[:], in_=var[:], func=AF.Sqrt, bias=eps_c[:, :1], scale=1.0 / D)
        rr = sb.tile([128, nchunks], F32, name="lnrr", tag="ln_rr")
        nc.vector.reciprocal(out=rr[:], in_=sd[:])
        an = sb.tile([128, nchunks, D], F32, name="lnan", tag="ln_an")
        nc.vector.tensor_tensor(out=an[:], in0=xc[:],
                                in1=rr[:].rearrange("p (c o) -> p c o", o=1).to_broadcast([128, nchunks, D]),
                                op=AL.mult)
        for r in range(nchunks):
            tp = psA.tile([128, 128], F32, space="PSUM", name="lntp2", tag="sA")
            nc.tensor.transpose(out=tp[:], in_=an[:, r, :], identity=idn[:])
            nc.scalar.activation(out=dst_T[:, 128 * r:128 * (r + 1)], in_=tp[:],
                                 func=AF.Copy, bias=0.0, scale=1.0)
        return an

    lnc_T = sb.tile([128, NH], F32R, name="lnc_T")
    ln_featpart(cond_T, 9, lnc_T)
    if STAGE < 2:
        outp0 = sb.tile([128, NBLK, D], F32, name="outp0")
        nc.vector.tensor_copy(out=outp0[:], in_=lnc_T[:, 0:1024].rearrange("p (c d) -> p c d", d=D).bitcast(F32))
        nc.sync.dma_start(out=out_d[:, :, :], in_=outp0[:])
        ctx.close()
        return

    # LN(q) directly from rows-part q
    qh = load("qh")
    mu_q = sb.tile([128, 9], F32)
    nc.vector.tensor_reduce(out=mu_q[:], in_=qh[:], axis=AX.X, op=AL.add)
    nc.vector.tensor_scalar(out=mu_q[:], in0=mu_q[:], scalar1=1.0 / D, scalar2=None, op0=AL.mult)
    xc_q = sb.tile([128, 9, D], F32, name="xc_q", tag="ln_xc")
    nc.vector.tensor_tensor(out=xc_q[:], in0=qh[:],
                            in1=mu_q[:].rearrange("p (c o) -> p c o", o=1).to_broadcast([128, 9, D]),
                            op=AL.subtract)
    sq_q = sb.tile([128, 9, D], F32, name="sq_q", tag="ln_sq")
    nc.vector.tensor_tensor(out=sq_q[:], in0=xc_q[:], in1=xc_q[:], op=AL.mult)
    var_q = sb.tile([128, 9], F32)
    nc.vector.tensor_reduce(out=var_q[:], in_=sq_q[:], axis=AX.X, op=AL.add)
    sd_q = sb.tile([128, 9], F32)
    nc.scalar.activation(out=sd_q[:], in_=var_q[:], func=AF.Sqrt, bias=eps_c[:, :1], scale=1.0 / D)
    rr_q = sb.tile([128, 9], F32)
    nc.vector.reciprocal(out=rr_q[:], in_=sd_q[:])
    an_q = sb.tile([128, 9, D], F32, name="an_q", tag="ln_an")
    nc.vector.tensor_tensor(out=an_q[:], in0=xc_q[:],
                            in1=rr_q[:].rearrange("p (c o) -> p c o", o=1).to_broadcast([128, 9, D]),
                            op=AL.mult)
    anq_T = sb.tile([128, NH], F32, name="anq_T")
    for r in range(9):
        tp = psA.tile([128, 128], F32, space="PSUM", name="aqtp", tag="sA")
        nc.tensor.transpose(out=tp[:], in_=an_q[:, r, :], identity=idn[:])
        nc.scalar.activation(out=anq_T[:, 128 * r:128 * (r + 1)], in_=tp[:],
                             func=AF.Copy, bias=0.0, scale=1.0)

    # adaln1: q_n = sigmoid(lnc@s1sw + s1sb) * a_n + lnc@s1bw   (feat-part)
    qn_T = sb.tile([128, NH], F16, name="qn_T")
    for t in range(3):
        sl = slice(384 * t, 384 * (t + 1))
        sc_ps = pj.tile([128, 512], F32, space="PSUM", name="sc_ps", tag="pj")
        nc.tensor.matmul(out=sc_ps[:, 0:384], lhsT=s1sw[:], rhs=lnc_T[:, sl], start=True, stop=True)
        scal = sb2.tile([128, 384], F32, name="scal", tag="scal")
        nc.scalar.activation(out=scal[:], in_=sc_ps[:, 0:384], func=AF.Sigmoid, bias=s1sb[:, :1], scale=1.0)
        bi_ps = pj.tile([128, 512], F32, space="PSUM", name="bi_ps", tag="pj")
        nc.tensor.matmul(out=bi_ps[:, 0:384], lhsT=s1bw[:], rhs=lnc_T[:, sl], start=True, stop=True)
        tmp = sb2.tile([128, 384], F32, name="qn_tmp", tag="qn_tmp")
        nc.vector.tensor_tensor(out=tmp[:], in0=scal[:], in1=anq_T[:, sl], op=AL.mult)
        nc.vector.tensor_tensor(out=qn_T[:, sl], in0=tmp[:], in1=bi_ps[:, 0:384], op=AL.add)

    # Q, K (feat-part, fp16), V (rows-part aug, fp16), G sigmoid (rows-part)
    Q_T = sb.tile([128, NH], F16, name="Q_T")
    K_T = sb.tile([128, NH], F16, name="K_T")
    for t in range(3):
        sl = slice(384 * t, 384 * (t + 1))
        qps = pj.tile([128, 512], F32, space="PSUM", name="qps", tag="pj")
        nc.tensor.matmul(out=qps[:, 0:384], lhsT=wqT[:], rhs=qn_T[:, sl], start=True, stop=True)
        nc.scalar.activation(out=Q_T[:, sl], in_=qps[:, 0:384], func=AF.Identity, bias=wqb[:, :1], scale=1.0)
        kps = pj.tile([128, 512], F32, space="PSUM", name="kps", tag="pj")
        nc.tensor.matmul(out=kps[:, 0:384], lhsT=wkT[:], rhs=qn_T[:, sl], start=True, stop=True)
        nc.scalar.activation(out=K_T[:, sl], in_=kps[:, 0:384], func=AF.Copy, bias=0.0, scale=1.0)

    v_aug = sb.tile([128, 9, H, 33], F16, name="v_aug")
    nc.vector.memset(v_aug[:, :, :, 32:33], 1.0)
    for r in range(9):
        vps = pj.tile([128, 512], F32, space="PSUM", name="vps", tag="pj")
        nc.tensor.matmul(out=vps[:, 0:128], lhsT=qn_T[:, 128 * r:128 * (r + 1)], rhs=wvT[:],
                         start=True, stop=True)
        nc.scalar.activation(out=v_aug[:, r, :, 0:32], in_=vps[:, 0:128].rearrange("p (h e) -> p h e", h=H),
                             func=AF.Copy, bias=0.0, scale=1.0)
    sigG = sb.tile([128, NBLK, D], F32, name="sigG")
    ag_sb = sb.tile([128, NBLK, D], F32, name="ag_sb")
    tg_sb = sb.tile([128, NBLK, D], F32, name="tg_sb")
    agTr = wload("agT", (D, D), F32R)
    tgTr = wload("tgT", (D, D), F32R)
    for i in range(NBLK):
        csl = slice(16 + 128 * i, 16 + 128 * (i + 1))
        gps = pj.tile([128, 512], F32, space="PSUM", name="gps", tag="pj")
        nc.tensor.matmul(out=gps[:, 0:128], lhsT=qn_T[:, csl], rhs=wgT[:], start=True, stop=True)
        nc.scalar.activation(out=sigG[:, i, :], in_=gps[:, 0:128], func=AF.Sigmoid, bias=0.0, scale=1.0)
        for wT, bb, dst in ((agTr, agb, ag_sb), (tgTr, tgb, tg_sb)):
            gp2 = pj.tile([128, 512], F32, space="PSUM", name="gp2", tag="pj")
            nc.tensor.matmul(out=gp2[:, 0:128], lhsT=cond_Tr[:, csl], rhs=wT[:], start=True, stop=True)
            gtmp = sb2.tile([128, 128], F32, name="gtmp", tag="gtmp")
            nc.vector.tensor_tensor(out=gtmp[:], in0=gp2[:, 0:128], in1=bb[:], op=AL.add)
            nc.scalar.activation(out=dst[:, i, :], in_=gtmp[:], func=AF.Sigmoid, bias=0.0, scale=1.0)

    if STAGE < 3:
        outp0 = sb.tile([128, NBLK, D], F32, name="outp0")
        nc.vector.tensor_copy(out=outp0[:], in_=sigG[:])
        nc.sync.dma_start(out=out_d[:, :, :], in_=outp0[:])
        ctx.close()
        return

    # ---------------- banded attention ----------------
    attn_o = sb.tile([128, NBLK, D], F32, name="attn_o")
    for i in range(NBLK):
        isl = slice(16 + 128 * i, 16 + 128 * (i + 1))
        slab1 = sb2.tile([128, 128, 4], F32, name="slab1", tag="slab1")
        dma1 = nc.sync.dma_start(
            out=slab1[:], in_=io["bandi"][20480 * i:20480 * i + 16384, :].rearrange("(a b) f -> a b f", a=128))
        slab2 = sb2.tile([32, 128, 4], F32, name="slab2", tag="slab2")
        dma2 = nc.sync.dma_start(
            out=slab2[:], in_=io["bandi"][20480 * i + 16384:20480 * (i + 1), :].rearrange("(a b) f -> a b f", a=32))
        den4 = sb2.tile([128, 4], F32, name="den4", tag="den4")
        for h in range(H):
            hs = slice(32 * h, 32 * (h + 1))
            tp = (32 * h, 0) if h else (0, 0)
            sA = psA.tile([128, 128], F32, space="PSUM", name="sA", tag="sA")
            nc.tensor.matmul(out=sA[:], lhsT=K_T[hs, 128 * i:128 * (i + 1)], rhs=Q_T[hs, isl],
                             start=True, stop=True, tile_position=tp)
            sB = psB.tile([32, 128], F32, space="PSUM", name="sB", tag="sB")
            nc.tensor.matmul(out=sB[:], lhsT=K_T[hs, 128 * (i + 1):128 * (i + 1) + 32], rhs=Q_T[hs, isl],
                             start=True, stop=True, tile_position=tp)
            eA = sb2.tile([128, 128], F16, name="eA", tag="eA")
            tA = sb2.tile([128, 128], F32, name="tA", tag="tA")
            nc.vector.tensor_tensor(out=tA[:], in0=sA[:], in1=slab1[:, :, h], op=AL.add)
            nc.scalar.activation(out=eA[:], in_=tA[:], func=AF.Exp, bias=shf_c[:, :1], scale=1.0)
            eB = sb2.tile([32, 128], F16, name="eB", tag="eB")
            tB = sb2.tile([32, 128], F32, name="tB", tag="tB")
            nc.vector.tensor_tensor(out=tB[:], in0=sB[:], in1=slab2[:, :, h], op=AL.add)
            nc.scalar.activation(out=eB[:], in_=tB[:], func=AF.Exp, bias=shf_c[0:32, :1], scale=1.0)
            ao = psO.tile([128, 33], F32, space="PSUM", name="ao", tag="ao")
            nc.tensor.matmul(out=ao[:], lhsT=eA[:], rhs=v_aug[:, i, h, :], start=True, stop=False)
            nc.tensor.matmul(out=ao[:], lhsT=eB[:], rhs=v_aug[0:32, i + 1, h, :], start=False, stop=True)
            nc.vector.tensor_copy(out=den4[:, h:h + 1], in_=ao[:, 32:33])
            nc.scalar.activation(out=attn_o[:, i, hs], in_=ao[:, 0:32], func=AF.Copy, bias=0.0, scale=1.0)
        rden = sb2.tile([128, 4], F32, name="rden", tag="rden")
        nc.vector.reciprocal(out=rden[:], in_=den4[:])
        nc.vector.tensor_tensor(out=attn_o[:, i, :].rearrange("p (h e) -> p h e", h=H),
                                in0=attn_o[:, i, :].rearrange("p (h e) -> p h e", h=H),
                                in1=rden[:].rearrange("p (h o) -> p h o", o=1).to_broadcast([128, H, 32]),
                                op=AL.mult)

    # gate by sigmoid(G), project wo, residual
    gated = attn_o
    nc.vector.tensor_tensor(out=gated[:], in0=attn_o[:], in1=sigG[:], op=AL.mult)
    qc = load("qc")
    q1 = sb.tile([128, NBLK, D], F32, name="q1")
    for i in range(NBLK):
        tp = psA.tile([128, 128], F32, space="PSUM", name="gtp", tag="sA")
        nc.tensor.transpose(out=tp[:], in_=gated[:, i, :], identity=idn[:])
        gT = sb2.tile([128, 128], F16, name="gT", tag="gT")
        nc.scalar.activation(out=gT[:], in_=tp[:], func=AF.Copy, bias=0.0, scale=1.0)
        ups = pj.tile([128, 512], F32, space="PSUM", name="ups", tag="pj")
        nc.tensor.matmul(out=ups[:, 0:128], lhsT=gT[:], rhs=woT[:], start=True, stop=True)
        ut1 = sb2.tile([128, 128], F32, name="ut1", tag="ut1")
        nc.vector.tensor_tensor(out=ut1[:], in0=ups[:, 0:128], in1=ag_sb[:, i, :], op=AL.mult)
        nc.vector.tensor_tensor(out=q1[:, i, :], in0=qc[:, i, :], in1=ut1[:], op=AL.add)

    if STAGE < 4:
        nc.sync.dma_start(out=out_d[:, :, :], in_=q1[:])
        ctx.close()
        return

    # adaln2 on q1 (rows-part stats), then swiglu
    mu1 = sb.tile([128, NBLK], F32)
    nc.vector.tensor_reduce(out=mu1[:], in_=q1[:], axis=AX.X, op=AL.add)
    nc.vector.tensor_scalar(out=mu1[:], in0=mu1[:], scalar1=1.0 / D, scalar2=None, op0=AL.mult)
    xc1 = sb.tile([128, NBLK, D], F32, name="xc1", tag="ln_xc")
    nc.vector.tensor_tensor(out=xc1[:], in0=q1[:],
                            in1=mu1[:].rearrange("p (c o) -> p c o", o=1).to_broadcast([128, NBLK, D]),
                            op=AL.subtract)
    sq1 = sb.tile([128, NBLK, D], F32, name="sq1", tag="ln_sq")
    nc.vector.tensor_tensor(out=sq1[:], in0=xc1[:], in1=xc1[:], op=AL.mult)
    var1 = sb.tile([128, NBLK], F32)
    nc.vector.tensor_reduce(out=var1[:], in_=sq1[:], axis=AX.X, op=AL.add)
    sd1 = sb.tile([128, NBLK], F32)
    nc.scalar.activation(out=sd1[:], in_=var1[:], func=AF.Sqrt, bias=eps_c[:, :1], scale=1.0 / D)
    rr1 = sb.tile([128, NBLK], F32)
    nc.vector.reciprocal(out=rr1[:], in_=sd1[:])
    an1 = sb.tile([128, NBLK, D], F32, name="an1", tag="ln_an")
    nc.vector.tensor_tensor(out=an1[:], in0=xc1[:],
                            in1=rr1[:].rearrange("p (c o) -> p c o", o=1).to_broadcast([128, NBLK, D]),
                            op=AL.mult)
    an1_T = sb.tile([128, RC], F32, name="an1_T")
    for r in range(NBLK):
        tp = psA.tile([128, 128], F32, space="PSUM", name="a1tp", tag="sA")
        nc.tensor.transpose(out=tp[:], in_=an1[:, r, :], identity=idn[:])
        nc.scalar.activation(out=an1_T[:, 128 * r:128 * (r + 1)], in_=tp[:],
                             func=AF.Copy, bias=0.0, scale=1.0)

    if STAGE < 5:
        nc.sync.dma_start(out=out_d[:, :, :], in_=an1[:])
        ctx.close()
        return

    qn2_T = sb.tile([128, RC], F16, name="qn2_T")
    for t in range(2):
        sl = slice(512 * t, 512 * (t + 1))
        csl = slice(16 + 512 * t, 16 + 512 * (t + 1))
        sc_ps = pj.tile([128, 512], F32, space="PSUM", name="sc2ps", tag="pj")
        nc.tensor.matmul(out=sc_ps[:], lhsT=s2sw[:], rhs=lnc_T[:, csl], start=True, stop=True)
        scal = sb2.tile([128, 512], F32, name="scal2", tag="scal2")
        nc.scalar.activation(out=scal[:], in_=sc_ps[:], func=AF.Sigmoid, bias=s2sb[:, :1], scale=1.0)
        bi_ps = pj.tile([128, 512], F32, space="PSUM", name="bi2ps", tag="pj")
        nc.tensor.matmul(out=bi_ps[:], lhsT=s2bw[:], rhs=lnc_T[:, csl], start=True, stop=True)
        tmp = sb2.tile([128, 512], F32, name="qn2tmp", tag="qn2tmp")
        nc.vector.tensor_tensor(out=tmp[:], in0=scal[:], in1=an1_T[:, sl], op=AL.mult)
        nc.vector.tensor_tensor(out=qn2_T[:, sl], in0=tmp[:], in1=bi_ps[:], op=AL.add)

    hid = sb.tile([128, 4, RC], F16, name="hid")
    for t in range(2):
        sl = slice(512 * t, 512 * (t + 1))
        for cidx in range(4):
            ups1 = pj.tile([128, 512], F32, space="PSUM", name="swu", tag="pj")
            nc.tensor.matmul(out=ups1[:], lhsT=w1T[:, 128 * cidx:128 * (cidx + 1)], rhs=qn2_T[:, sl],
                             start=True, stop=True)
            uu = sb2.tile([128, 512], F32, name="uu", tag="uu")
            nc.scalar.activation(out=uu[:], in_=ups1[:], func=AF.Silu, bias=0.0, scale=1.0)
            vps1 = pj.tile([128, 512], F32, space="PSUM", name="swv", tag="pj")
            nc.tensor.matmul(out=vps1[:], lhsT=w3T[:, 128 * cidx:128 * (cidx + 1)], rhs=qn2_T[:, sl],
                             start=True, stop=True)
            nc.vector.tensor_tensor(out=hid[:, cidx, sl], in0=uu[:], in1=vps1[:], op=AL.mult)

    if STAGE < 6:
        nc.sync.dma_start(out=out_d[:, :, :], in_=q1[:])
        ctx.close()
        return

    outp = q1
    for i in range(NBLK):
        rsl = slice(128 * i, 128 * (i + 1))
        w2ps = pj.tile([128, 512], F32, space="PSUM", name="w2ps", tag="pj")
        for cidx in range(4):
            nc.tensor.matmul(out=w2ps[:, 0:128], lhsT=hid[:, cidx, rsl], rhs=w2T[:, cidx, :],
                             start=(cidx == 0), stop=(cidx == 3))
        ut2 = sb2.tile([128, 128], F32, name="ut2", tag="ut2")
        nc.vector.tensor_tensor(out=ut2[:], in0=w2ps[:, 0:128], in1=tg_sb[:, i, :], op=AL.mult)
        nc.vector.tensor_tensor(out=outp[:, i, :], in0=q1[:, i, :], in1=ut2[:], op=AL.add)
    nc.vector.tensor_tensor(out=outp[:], in0=outp[:],
                            in1=pad[:].rearrange("p (c o) -> p c o", o=1).to_broadcast([128, NBLK, D]),
                            op=AL.mult)
    nc.sync.dma_start(out=out_d[:, :, :], in_=outp[:])
    ctx.close()


def _host_prep(inputs):
    """Build per-core in_maps from full inputs."""
    p = inputs["params"]
    q = np.asarray(inputs["q"], np.float32)
    hc = np.asarray(inputs["h_cond"], np.float32)
    plm = np.asarray(inputs["p_lm"], np.float32)
    pli = np.asarray(inputs["p_lm_idx"], np.int64)
    temb = np.asarray(inputs["t_emb"], np.float32)
    tok = np.asarray(inputs["token_idx"], np.int64)
    apm = np.asarray(inputs["atom_pad_mask"], np.float32)
    pvm = np.asarray(inputs["pair_valid_mask"], np.float32)

    f32 = lambda x: np.ascontiguousarray(x, np.float32)
    sq32 = float(np.sqrt(DH))

    # shared weights
    sw = {
        "cwT": f32(np.asarray(p["cond_w"]).T),
        "cb": f32(np.asarray(p["cond_b"])[:, None]),
        "wqb": f32((np.asarray(p["wq_b"]) / sq32)[:, None]),
        "wqT": f32(np.asarray(p["wq_w"]).T / sq32),
        "wkT": f32(np.asarray(p["wk_w"]).T),
        "wvT": f32(np.asarray(p["wv_w"]).T),
        "wgT": f32(np.asarray(p["wg_w"]).T),
        "woT": f32(np.asarray(p["wo_w"]).T),
        "agT": f32(np.asarray(p["attn_gate_w"]).T),
        "tgT": f32(np.asarray(p["trans_gate_w"]).T),
        "agb": f32(np.broadcast_to(np.asarray(p["attn_gate_b"]), (128, 128))),
        "tgb": f32(np.broadcast_to(np.asarray(p["trans_gate_b"]), (128, 128))),
        "w1T": f32(np.asarray(p["swiglu_w1"]).T),
        "w3T": f32(np.asarray(p["swiglu_w3"]).T),
        "w2T": f32(np.asarray(p["swiglu_w2"]).T),
        "ut": np.triu(np.ones((128, 128), np.float32), 1),
        "iota": np.tile(np.arange(128, dtype=np.float32), (128, 1)),
        "gval": f32(np.arange(128)[:, None] * 16.0 + np.arange(16)[None, :]),
        "idn": np.eye(128, dtype=np.float32),
    }
    for s, ad in (("s1", p["adaln1"]), ("s2", p["adaln2"])):
        w = np.asarray(ad["s_norm_w"], np.float32)
        sw[s + "sw"] = f32((np.asarray(ad["scale_w"]) * w[None, :]).T)
        sw[s + "bw"] = f32((np.asarray(ad["bias_w"]) * w[None, :]).T)
        sw[s + "sb"] = f32(np.asarray(ad["scale_b"])[:, None])
    pw = np.asarray(p["pair_bias_w"], np.float32) * np.asarray(p["ln_pair_w"], np.float32)[None, :]
    sw["pbw"] = f32(np.broadcast_to(pw[None], (128, H, 16)))
    sw["pbs"] = f32(np.broadcast_to(pw.sum(1)[None], (128, H)))
    pc = (np.asarray(p["pair_bias_w"], np.float32) * np.asarray(p["ln_pair_b"], np.float32)[None, :]).sum(1)
    sw["pbc"] = f32(np.broadcast_to(pc[None], (128, H)))

    in_maps = []
    for c in range(8):
        b, ch = c // 4, c % 4
        r0 = RC * ch
        m = dict(sw)
        # halo q rows [r0-16, r0+1136) clamped
        hr = np.clip(np.arange(r0 - 16, r0 - 16 + NH), 0, N - 1)
        m["qh"] = f32(q[b][hr].reshape(9, 128, D).transpose(1, 0, 2))
        m["qc"] = f32(q[b, r0:r0 + RC].reshape(NBLK, 128, D).transpose(1, 0, 2))
        m["hc"] = f32(hc[b])
        m["tok"] = np.ascontiguousarray(tok[b][hr].reshape(9, 128).T.astype(np.int32))
        m["tv"] = f32(temb[b][:, None])
        m["pi"] = np.ascontiguousarray(pli[b, :, 0].reshape(128, 128).astype(np.int32))
        m["pj"] = np.ascontiguousarray(pli[b, :, 1].reshape(128, 128).astype(np.int32))
        m["pv"] = np.ascontiguousarray((pvm[b] > 0).reshape(128, 128).astype(np.int32))
        m["plm"] = f32(plm[b].reshape(128, 128, 16))
        m["rvec"] = np.full((128, 1), r0, np.int32)
        m["pad"] = f32(apm[b, r0:r0 + RC].reshape(NBLK, 128).T)
        # band mask init: slot = I*JW*D + jl*D + il ; value 0 if in-window & j valid
        Ii = np.arange(NBLK)[:, None, None]
        jli = np.arange(JW)[None, :, None]
        ili = np.arange(D)[None, None, :]
        jg = r0 - 16 + 128 * Ii + jli
        ok = (jli >= ili) & (jli <= ili + 2 * WIN) & (jg >= 0) & (jg < N)
        bm = np.where(ok, 0.0, -1e30).astype(np.float32)
        m["bandi"] = f32(np.repeat(bm.reshape(NSLOT, 1), 4, axis=1))
        in_maps.append(m)
    return in_maps


def kernel(**inputs):
    global _PROG
    if _PROG is None:
        _PROG = _build_program()
    nc = _PROG
    in_maps = _host_prep(inputs)
    res = run_bass_kernel_spmd(nc, in_maps, list(range(8))).results
    out = np.empty((B, N, D), np.float32)
    for c in range(8):
        b, ch = c // 4, c % 4
        o = res[c]["out"]  # (128, 8, 128)
        out[b, RC * ch:RC * (ch + 1)] = o.transpose(1, 0, 2).reshape(RC, D)
    return out
